# revision 21
# baseline (speedup 1.0000x reference)
"""Causal self-attention Trainium2 Bass kernel (v6).

Problem: B=2, N=2048, D=1024, H=16 heads, DH=64 (fp32).
  kqv = einsum('bnd,hed->bhne', x, Wqkv) + bqkv   (chunk order k, q, v)
  scores = q @ k^T / 8, causal mask, softmax
  sa = attn @ v, concat heads, out = sa @ Wproj.T + bproj

Sharding (8 cores): data-parallel over B (2) x tensor-parallel over heads
(4 heads/core).  Each core computes its 4 heads' contribution to the proj
output for its batch; the host sums the 4 partials per batch and adds
bproj (the "all-reduce after proj" done host-side during unsharding).

Design (v6; v3 baseline was 181-205us):
  - Scores: the contraction is only DH=64, so the two heads of a
    head-pair run as two CONCURRENT K=64 matmuls in different PE row
    groups (tile_position (0,0)/(64,0) derives from the operand base
    partitions) writing different PSUM banks -> 2x score throughput.
    The e-layout packs each head-pair's even head in rows 0:64 and odd
    head in rows 64:128 of the same slab, so k and q slices line up.
  - Everything bf16.  fp8 DoubleRow was tried (v4/v5) and measured on
    HW: a DR matmul streams both k-subtiles serially (~634ns vs 2x216ns
    for the bf16 pair), so fp8 lost time AND accuracy.
  - Diagonal-block column trim: score MMs, exp and PV only touch
    q-columns >= each m-tile's causal start (~15% of attention work).
  - One fused unit per (head-pair, q-block); m-tile granular pipeline:
    scores 2 tiles ahead, exp on ScalarE, causal staircase on GpSimd,
    PV per m-tile, softmax normalize on DVE, per-q-block projection
    interleaved as soon as its saT slab completes.
  - PV ones-column trick: V operand columns 64:128 are ones, so the PV
    matmul emits the softmax denominator for free.
  - V transpose on the PE (DMA-xbar transpose measured 1.2us/tile: far
    too slow).  Transpose psum tiles borrow the QKV psum tag; they are
    only emitted at qkv-group boundaries so the in-order PE queue can
    never deadlock against the pool's slot-free semaphore.
  - Output DMA'd as bf16 (host accumulates partials in fp32).
"""

import numpy as np
from contextlib import ExitStack

B, N, D, H = 2, 2048, 1024, 16
DH = 64
NH = 4                    # heads per core
HP = 2                    # head-pairs per core
ET = 6                    # e-slabs: [v01 k01 q01 | v23 k23 q23]
DT = D // 128             # 8 d-tiles (contraction)
NBS = 512                 # n block size (q-block width)
NB = N // NBS             # 4 n blocks
MTS = 128                 # m tile size (key-axis tile)
MT = N // MTS             # 16 m tiles
KT = NH * DH // 128       # 2 proj contraction tiles (256 local d_in)

_CACHE = {}


def _build_nc():
    import concourse.mybir as mybir
    import concourse.tile as tile
    from concourse import bacc

    f32 = mybir.dt.float32
    bf16 = mybir.dt.bfloat16
    EXP = mybir.ActivationFunctionType.Exp

    nc = bacc.Bacc("TRN2")
    xT_d = nc.dram_tensor("xT", [128, DT * N], bf16, kind="ExternalInput")
    wT_d = nc.dram_tensor("wT", [128, ET * DT * 128], bf16,
                          kind="ExternalInput")
    bq_d = nc.dram_tensor("bq", [128, ET], f32, kind="ExternalInput")
    wpT_d = nc.dram_tensor("wpT", [128, KT * D], bf16,
                           kind="ExternalInput")
    out_d = nc.dram_tensor("outp", [N, D], bf16, kind="ExternalOutput")

    xTr = xT_d.rearrange("p (t n) -> p t n", t=DT)
    wTr = wT_d.rearrange("p (e t j) -> p e t j", e=ET, t=DT)
    wpTr = wpT_d.rearrange("p (k f) -> p k f", k=KT)

    with tile.TileContext(nc) as tc, ExitStack() as ctx:
        const = ctx.enter_context(tc.tile_pool(name="const", bufs=1))
        xp = ctx.enter_context(tc.tile_pool(name="xw", bufs=1))
        qps = ctx.enter_context(tc.tile_pool(name="qps", bufs=2,
                                             space="PSUM"))
        sps = ctx.enter_context(tc.tile_pool(name="sps", bufs=2,
                                             space="PSUM"))
        pts = ctx.enter_context(tc.tile_pool(name="pts", bufs=4))
        sapp = ctx.enter_context(tc.tile_pool(name="sap", bufs=1,
                                              space="PSUM"))
        rrp = ctx.enter_context(tc.tile_pool(name="rrp", bufs=3))
        ost = ctx.enter_context(tc.tile_pool(name="ost", bufs=4))

        bq = const.tile([128, ET], f32)
        wpT = const.tile([128, KT, D], bf16)
        wst = const.tile([128, ET, DT, 128], bf16)
        xT = xp.tile([128, DT, N], bf16)
        # k/q bf16 tiles: [k01 q01 k23 q23]; head-even rows 0:64,
        # head-odd rows 64:128 (enables the K=64 split-row-group scores)
        kqv = const.tile([128, 4, N], bf16)
        vsrc = const.tile([128, HP, N], bf16)      # v01, v23 (pre-transp)
        # V^T + ones; the ones make PV emit the softmax denominator free
        vaug = const.tile([128, NH, MT, 128], bf16)
        saT = const.tile([128, KT, N], bf16)  # sa^T, local d_in rows

        # ---------------- input DMA schedule ----------------
        # DMAs are issued before any engine compute so the rings start
        # immediately after the framework preamble.
        # gpsimd ring: weights + bias; sync ring: even x dt chunks
        # (+ out DMAs later); scalar ring: odd x dt chunks.
        nc.gpsimd.dma_start(out=wst[:, 0:1], in_=wTr[:, 0:1])
        nc.gpsimd.dma_start(out=wst[:, 1:3], in_=wTr[:, 1:3])
        for dt in range(DT):
            eng = nc.sync if dt % 2 == 0 else nc.scalar
            eng.dma_start(out=xT[:, dt, :], in_=xTr[:, dt, :])
        nc.gpsimd.dma_start(out=bq, in_=bq_d[:, :])

        def fetch_aux_w():
            nc.gpsimd.dma_start(out=wst[:, 3:6], in_=wTr[:, 3:6])

        def fetch_wpT():
            nc.gpsimd.dma_start(out=wpT, in_=wpTr)

        # ---------------- one-time SBUF constants ----------------
        # preload the exp table set with a dummy ACTIVATE (~2.7us once)
        dum0 = const.tile([128, 1], bf16)
        nc.gpsimd.memset(dum0, 0.0)
        nc.scalar.activation(saT[:, 0, 0:1], dum0, EXP)
        nc.gpsimd.memset(vaug[:, :, :, DH:128], 1.0)
        # bacc pre-registers const APs; the BIR verifier rejects unread
        # SBUF - give the unused ones readers (slots overwritten later)
        for i, key in enumerate([(f32, 1.0), (bf16, 1.0),
                                 (mybir.dt.uint8, 127)]):
            nc.vector.tensor_copy(saT[:, 0, i + 1:i + 2],
                                  nc.const_aps.aps[key])
        # identity for PE-mode transpose
        ones = const.tile([128, 128], bf16)
        nc.gpsimd.memset(ones, 1.0)
        ident = const.tile([128, 128], bf16)
        nc.gpsimd.affine_select(
            ident, ones, pattern=[[-1, 128]], base=0,
            channel_multiplier=1,
            compare_op=mybir.AluOpType.is_equal, fill=0.0)

        # ---------------- QKV building blocks ----------------
        # e-slab -> destination: v->vsrc, k/q->kqv
        def qkv_dst(et):
            return {0: vsrc[:, 0, :], 1: kqv[:, 0, :], 2: kqv[:, 1, :],
                    3: vsrc[:, 1, :], 4: kqv[:, 2, :], 5: kqv[:, 3, :]}[et]

        def qkv_mm(et, nbh, dt, pss):
            for nb in (2 * nbh, 2 * nbh + 1):
                nc.tensor.matmul(
                    pss[nb % 2],
                    lhsT=wst[:, et, dt, :],
                    rhs=xT[:, dt, nb * NBS:(nb + 1) * NBS],
                    start=(dt == 0),
                    stop=(dt == DT - 1),
                )

        def qkv_bias(et, nb, ps):
            nc.vector.tensor_scalar_add(
                out=qkv_dst(et)[:, nb * NBS:(nb + 1) * NBS],
                in0=ps,
                scalar1=bq[:, et:et + 1],
            )

        def qkv_group_items(et):
            """Closures for one e-slab's projection."""
            items = []
            for nbh in range(2):
                holder = {}

                def alloc(et=et, holder=holder):
                    holder["pss"] = [
                        qps.tile([128, NBS], f32, tag="qkvps",
                                 name=f"qkvps{et}_{i}")
                        for i in range(2)]
                items.append(alloc)
                for dt in range(DT):
                    items.append(lambda et=et, nbh=nbh, dt=dt,
                                 holder=holder:
                                 qkv_mm(et, nbh, dt, holder["pss"]))
                for i in range(2):
                    items.append(lambda et=et, nbh=nbh, i=i, holder=holder:
                                 qkv_bias(et, 2 * nbh + i,
                                          holder["pss"][i]))
            return items

        def vtrans_items(vt):
            """V transpose on the PE + 2-head copy into vaug.

            NOTE: each item allocates a psum tile from the qkv tag; items
            must only run at qkv-group boundaries (aux list order
            guarantees this) or the in-order PE queue deadlocks waiting
            for a slot freed by a later instruction.
            """
            items = []

            def tr(mt, vt=vt):
                pv = qps.tile([128, 128], bf16, tag="qkvps", name="pv")
                with nc.allow_low_precision(reason="transpose pass"):
                    nc.tensor.transpose(
                        pv, vsrc[:, vt, mt * MTS:(mt + 1) * MTS], ident)
                nc.scalar.copy(vaug[:, 2 * vt:2 * vt + 2, mt, 0:DH], pv)

            for mt0 in range(0, MT, 2):
                def grp(mt0=mt0, vt=vt):
                    tr(mt0, vt)
                    tr(mt0 + 1, vt)
                items.append(grp)
            return items

        # ---------------- serial prefix: v01, k01, q01 ----------------
        for it in qkv_group_items(0):
            it()
        for it in vtrans_items(0)[:2]:      # vaug mts 0..3 for first PVs
            it()
        for et in (1, 2):
            for it in qkv_group_items(et):
                it()

        # aux work paced into the attention pipeline.  Transpose items
        # sit between complete qkv groups (see vtrans_items note).
        aux = vtrans_items(0)[2:]
        aux.append(fetch_aux_w)
        aux.extend(qkv_group_items(3))
        aux.extend(vtrans_items(1))
        aux.extend(qkv_group_items(4))
        aux.extend(qkv_group_items(5))
        aux.append(fetch_wpT)

        # ---------------- attention + projection pipeline ----------------
        # unit = (head-pair, q-block); g = (unit, m-tile)
        units = []
        for hp in (0, 1):
            for qb in (3, 2, 1, 0):
                units.append((hp, qb))
        gp = []
        for ui, (hp, qb) in enumerate(units):
            for mt in range(4 * qb + 4):
                gp.append((ui, mt))
        TOT = len(gp)                 # 80

        sap_tiles = {}
        pt_tiles = {}
        state = {"s": 0}

        def emit_S(g):
            ui, mt = gp[g]
            hp, qb = units[ui]
            qsl = slice(qb * NBS, (qb + 1) * NBS)
            msl = slice(mt * MTS, (mt + 1) * MTS)
            diag = mt >= 4 * qb
            qlo = (mt - 4 * qb) * MTS if diag else 0
            pt = pts.tile([128, 2, NBS], bf16, tag="pt", name="pt")
            pt_tiles[g] = pt
            sp = sps.tile([128, 2, NBS], f32, tag="sp", name="sp")
            for j in range(2):
                pb = slice(64 * j, 64 * j + 64)
                nc.tensor.matmul(
                    sp[:, j, qlo:],
                    lhsT=kqv[pb, 2 * hp, msl],
                    rhs=kqv[pb, 2 * hp + 1, qsl][:, qlo:],
                    start=True, stop=True,
                )
            if not diag:
                nc.scalar.activation(pt, sp, EXP, scale=0.125)
            else:
                if qlo + MTS < NBS:
                    nc.scalar.activation(pt[:, :, qlo + MTS:],
                                         sp[:, :, qlo + MTS:],
                                         EXP, scale=0.125)
                scr = pts.tile([128, 2, MTS], bf16, tag="scr", name="scr")
                nc.scalar.activation(scr, sp[:, :, qlo:qlo + MTS],
                                     EXP, scale=0.125)
                # causal staircase on GpSimd: keep where q-col >= m-row
                nc.gpsimd.affine_select(
                    pt[:, :, qlo:qlo + MTS], scr,
                    pattern=[[0, 2], [1, MTS]], base=0,
                    channel_multiplier=-1,
                    compare_op=mybir.AluOpType.is_ge, fill=0.0)

        def pump_S(upto):
            while state["s"] <= min(upto, TOT - 1):
                emit_S(state["s"])
                state["s"] += 1

        def emit_PV(g):
            ui, mt = gp[g]
            hp, qb = units[ui]
            diag = mt >= 4 * qb
            qlo = (mt - 4 * qb) * MTS if diag else 0
            if mt == 0:
                sap_tiles[ui] = sapp.tile([128, 2, NBS], f32, tag="sap",
                                          name="sap")
            for j in range(2):
                nc.tensor.matmul(
                    sap_tiles[ui][:, j, qlo:],
                    lhsT=vaug[:, 2 * hp + j, mt, :],
                    rhs=pt_tiles[g][:, j, qlo:],
                    start=(mt == 0), stop=(mt == 4 * qb + 3),
                )
            del pt_tiles[g]

        def emit_norm(ui):
            hp, qb = units[ui]
            qsl = slice(qb * NBS, (qb + 1) * NBS)
            for j in range(2):
                sap = sap_tiles[ui][:, j, :]
                # HW constraints (micro-tested): reciprocal_approx_fast
                # only works at base partition 0, and 2-input DVE ops
                # need equal input base partitions.
                den = rrp.tile([128, NBS], f32, tag="den", name="den")
                nc.vector.tensor_copy(den[0:DH, :], sap[DH:128, :])
                rr = rrp.tile([128, NBS], f32, tag="rr", name="rr")
                nc.vector.reciprocal_approx_fast(
                    out=rr[0:DH, :], in_=den[0:DH, :])
                nc.vector.tensor_mul(
                    saT[64 * j:64 * j + 64, hp, qsl],
                    sap[0:DH, :], rr[0:DH, :])
            del sap_tiles[ui]

        def emit_proj(nt, borrow_sp=False):
            nsl = slice(nt * 128, (nt + 1) * 128)
            if borrow_sp:
                po = sps.tile([128, 2, NBS], f32, tag="sp", name="sp")
                po0, po1 = po[:, 0, :], po[:, 1, :]
            else:
                po0 = qps.tile([128, NBS], f32, tag="qkvps", name="po0")
                po1 = qps.tile([128, NBS], f32, tag="qkvps", name="po1")
            for kt in range(KT):
                lt = saT[:, kt, nsl]
                nc.tensor.matmul(po0, lhsT=lt, rhs=wpT[:, kt, 0:NBS],
                                 start=(kt == 0), stop=(kt == KT - 1))
                nc.tensor.matmul(po1, lhsT=lt, rhs=wpT[:, kt, NBS:D],
                                 start=(kt == 0), stop=(kt == KT - 1))
            ot = ost.tile([128, D], bf16, name="ot")
            nc.vector.tensor_copy(ot[:, 0:NBS], po0)
            # mid-loop: keep ScalarE free for exp; tail: ACT is idle
            if borrow_sp:
                nc.scalar.copy(ot[:, NBS:], po1)
            else:
                nc.vector.tensor_copy(ot[:, NBS:], po1)
            nc.sync.dma_start(out=out_d[nsl, :], in_=ot)

        pump_S(1)
        hp1_start = next(i for i, (ui, mt) in enumerate(gp)
                         if units[ui][0] == 1)
        for g in range(TOT):
            ui, mt = gp[g]
            hp, qb = units[ui]
            if g + 2 == hp1_start:
                # pump_S is about to emit hp1 scores, which read
                # k23/q23: every pending aux item is a prerequisite -
                # drain them all so writers are emitted before readers
                while aux:
                    aux.pop(0)()
            pump_S(g + 2)
            emit_PV(g)
            budget = 2
            while aux and budget > 0:
                aux.pop(0)()
                budget -= 1
            if mt == 4 * qb + 3:      # last m-tile of this unit
                emit_norm(ui)
                if hp == 1 and qb != 0:   # qb's saT slab is complete
                    aux.extend(
                        lambda nt=4 * qb + k: emit_proj(nt)
                        for k in range(4))
        while aux:
            aux.pop(0)()
        # tail: proj for the last finished qb (qb=0), borrowing the
        # drained score psum for buffering
        for k in range(4):
            emit_proj(k, borrow_sp=True)

    nc.compile()
    return nc


def _host_inputs(x, Wqkv, bqkv, Wproj):
    """Per-core input maps (host-side sharding + relayout, bf16 cast).

    All tensors are packed partition-major so every DMA descriptor is a
    contiguous row chunk.
    """
    import ml_dtypes
    bf16 = ml_dtypes.bfloat16

    in_maps = []
    for c in range(8):
        b, hg = c // NH, c % NH
        h0 = hg * NH
        # xT[p, dt, n] = x[b][n, dt*128+p]
        xT = np.ascontiguousarray(
            x[b].T.reshape(DT, 128, N).transpose(1, 0, 2)
            .reshape(128, DT * N)).astype(bf16)
        # e-slab order [v01 k01 q01 | v23 k23 q23]; within a slab the
        # head-pair's even head occupies rows 0:64, odd head 64:128.
        wq = Wqkv[h0:h0 + NH].reshape(NH, 3, DH, D)
        slabs = []
        bslabs = []
        bqc = bqkv[h0:h0 + NH].reshape(NH, 3, DH)
        for p2 in (0, 1):
            for comp in (2, 0, 1):            # v, k, q
                slabs.append(wq[2 * p2:2 * p2 + 2, comp].reshape(128, D))
                bslabs.append(bqc[2 * p2:2 * p2 + 2, comp].reshape(128))
        wre = np.concatenate(slabs, 0)        # [768, D]
        wT = wre.T                            # [D, 768]
        # wT2[p, et, dt, j] = wT[dt*128+p, et*128+j]
        wT2 = np.ascontiguousarray(
            wT.reshape(DT, 128, ET, 128).transpose(1, 2, 0, 3)
            .reshape(128, ET * DT * 128)).astype(bf16)
        bq2 = np.ascontiguousarray(
            np.stack(bslabs, axis=1)).astype(np.float32)   # [128, ET]
        wpT = Wproj[:, h0 * DH:(h0 + NH) * DH].T           # [256, D]
        wpT2 = np.ascontiguousarray(
            wpT.reshape(KT, 128, D).transpose(1, 0, 2)
            .reshape(128, KT * D)).astype(bf16)
        in_maps.append({"xT": xT, "wT": wT2, "bq": bq2, "wpT": wpT2})
    return in_maps


def _get_nc():
    if "nc" not in _CACHE:
        _CACHE["nc"] = _build_nc()
    return _CACHE["nc"]


def run_on_hw(in_maps, trace=False, **kw):
    from concourse.bass_utils import run_bass_kernel_spmd
    nc = _get_nc()
    return run_bass_kernel_spmd(
        nc, in_maps, core_ids=list(range(8)), trace=trace, **kw)


def kernel(**inputs):
    x = np.asarray(inputs["x"], dtype=np.float32)
    Wqkv = np.asarray(inputs["Wqkv"], dtype=np.float32)
    bqkv = np.asarray(inputs["bqkv"], dtype=np.float32)
    Wproj = np.asarray(inputs["Wproj"], dtype=np.float32)
    bproj = np.asarray(inputs["bproj"], dtype=np.float32)

    in_maps = _host_inputs(x, Wqkv, bqkv, Wproj)
    res = run_on_hw(in_maps).results

    out = np.zeros((B, N, D), dtype=np.float32)
    for b in range(B):
        acc = res[b * NH + 0]["outp"].astype(np.float32)
        for g in range(1, NH):
            acc = acc + res[b * NH + g]["outp"].astype(np.float32)
        out[b] = acc + bproj[None, :]
    return out


# revision 22
# speedup vs baseline: 1.1042x; 1.1042x over previous
"""Causal self-attention Trainium2 Bass kernel (v5).

Problem: B=2, N=2048, D=1024, H=16 heads, DH=64 (fp32).
  kqv = einsum('bnd,hed->bhne', x, Wqkv) + bqkv   (chunk order k, q, v)
  scores = q @ k^T / 8, causal mask, softmax
  sa = attn @ v, concat heads, out = sa @ Wproj.T + bproj

Sharding (8 cores): data-parallel over B (2) x tensor-parallel over heads
(4 heads/core).  Each core computes its 4 heads' contribution to the proj
output for its batch; the host sums the 4 partials per batch and adds
bproj (the "all-reduce after proj" done host-side during unsharding).

Design (v5, from v3's 181-205us; measured 173.6us traced):
  - QKV mostly in fp8(e4m3) with MatmulPerfMode.DoubleRow: 2 contraction
    k-tiles per instruction (weights pre-scaled x16 on the host, rescaled
    1/16 during the psum->SBUF bias move).  Halves the x input DMA too,
    which dominates the warmup prefix.  (A DR matmul streams its two
    k-subtiles serially on this silicon, so DR is NOT a 2x stream win -
    but halving the instruction count and the x DMA still nets out
    faster than the all-bf16 variant, which measured 195.5us.)
  - Accuracy carve-outs (rel err would blow past 2e-2 otherwise, measured
    in CoreSim): the n[0:512] block of k/q/v is computed in bf16 (few-key
    early q-rows don't average fp8 noise away), the qb=0 units run their
    PV in bf16, and the entire projection (saT + Wproj) stays bf16.
  - Scores: contraction is only DH=64, so the two heads of a head-pair
    run as two CONCURRENT K=64 matmuls in different PE row groups
    (tile_position (0,0)/(64,0) via operand base partitions) writing
    different PSUM banks -> 2x score throughput.
  - PV in fp8 DoubleRow over m-tile pairs (exp ACT writes pt as fp8).
  - exp(s/8 - 2): the -2 keeps exp inside fp8(e4m3) range; the constant
    factor cancels between the PV numerator and the ones-column
    denominator.
  - Diagonal-block column trim on score MMs / exp / PV (~15% of
    attention work).
  - V transpose on the PE (DMA-xbar transpose measured 1.2us per tile -
    far too slow).  Transpose psum tiles borrow the QKV psum tag; they
    are only emitted at group boundaries so the in-order PE queue can
    never deadlock against the pool's slot-free semaphore.
  - Output DMA'd as bf16 (host accumulates partials in fp32).
"""

import numpy as np
from contextlib import ExitStack

B, N, D, H = 2, 2048, 1024, 16
DH = 64
NH = 4                    # heads per core
HP = 2                    # head-pairs per core
ET = 6                    # e-slabs: [v01 k01 q01 | v23 k23 q23]
DT = D // 128             # 8 d-tiles (contraction)
DP = DT // 2              # 4 DoubleRow dt-pairs
NBS = 512                 # n block size (q-block width)
NB = N // NBS             # 4 n blocks
MTS = 128                 # m tile size (key-axis tile)
MT = N // MTS             # 16 m tiles
KT = NH * DH // 128       # 2 proj contraction tiles (256 local d_in)

_CACHE = {}


def _build_nc():
    import concourse.mybir as mybir
    import concourse.tile as tile
    from concourse import bacc

    f32 = mybir.dt.float32
    bf16 = mybir.dt.bfloat16
    f8 = mybir.dt.float8e4
    EXP = mybir.ActivationFunctionType.Exp
    DR = mybir.MatmulPerfMode.DoubleRow
    MULT = mybir.AluOpType.mult
    ADD = mybir.AluOpType.add

    nc = bacc.Bacc("TRN2")
    xT_d = nc.dram_tensor("xT", [128, DT * N], f8, kind="ExternalInput")
    wT_d = nc.dram_tensor("wT", [128, ET * DT * 128], f8,
                          kind="ExternalInput")
    bq_d = nc.dram_tensor("bq", [128, ET], f32, kind="ExternalInput")
    wpT_d = nc.dram_tensor("wpT", [128, KT * D], bf16,
                           kind="ExternalInput")
    x16_d = nc.dram_tensor("x16", [128, DT * NBS], bf16,
                           kind="ExternalInput")
    wv16_d = nc.dram_tensor("wv16", [128, ET * DT * 128], bf16,
                            kind="ExternalInput")
    out_d = nc.dram_tensor("outp", [N, D], bf16, kind="ExternalOutput")

    xTr = xT_d.rearrange("p (t n) -> p t n", t=DT)
    wTr = wT_d.rearrange("p (e t j) -> p e t j", e=ET, t=DT)
    wpTr = wpT_d.rearrange("p (k f) -> p k f", k=KT)
    x16r = x16_d.rearrange("p (t n) -> p t n", t=DT)
    wv16r = wv16_d.rearrange("p (v t j) -> p v t j", v=ET, t=DT)

    with tile.TileContext(nc) as tc, ExitStack() as ctx:
        const = ctx.enter_context(tc.tile_pool(name="const", bufs=1))
        xp = ctx.enter_context(tc.tile_pool(name="xw", bufs=1))
        qps = ctx.enter_context(tc.tile_pool(name="qps", bufs=2,
                                             space="PSUM"))
        sps = ctx.enter_context(tc.tile_pool(name="sps", bufs=2,
                                             space="PSUM"))
        pts = ctx.enter_context(tc.tile_pool(name="pts", bufs=4))
        sapp = ctx.enter_context(tc.tile_pool(name="sap", bufs=1,
                                              space="PSUM"))
        rrp = ctx.enter_context(tc.tile_pool(name="rrp", bufs=3))
        ost = ctx.enter_context(tc.tile_pool(name="ost", bufs=4))

        # preload the exp table set with a dummy ACTIVATE (~2.7us once)
        dum0 = const.tile([128, 1], bf16)
        nc.gpsimd.memset(dum0, 0.0)

        bq = const.tile([128, ET], f32)
        # exp(s/8 - 2) bias operand (see module docstring)
        nbias = const.tile([128, 1], f32)
        nc.gpsimd.memset(nbias, -2.0)
        wpT = const.tile([128, KT, D], bf16)
        wst = const.tile([128, ET, DT, 128], f8)
        xT = xp.tile([128, DT, N], f8)
        # bf16 copies for the n[0:512] block of k/q/v (accuracy carve-out)
        x16 = xp.tile([128, DT, NBS], bf16)
        wv16 = const.tile([128, ET, DT, 128], bf16)
        # k/q bf16 tiles: [k01 q01 k23 q23]; head-even rows 0:64,
        # head-odd rows 64:128 (enables the K=64 split-row-group scores)
        kqv = const.tile([128, 4, N], bf16)
        vsrc = const.tile([128, HP, N], bf16)      # v01, v23 (pre-transp)
        # bf16 V + ones for the qb=0 units' PV (mts 0..3 only needed)
        vaug16 = const.tile([128, NH, 4, 128], bf16)
        nc.gpsimd.memset(vaug16[:, :, :, DH:128], 1.0)
        # fp8 V + ones: the ones make the PV matmul emit the softmax
        # denominator for free
        vaug = const.tile([128, NH, MT, 128], f8)
        nc.gpsimd.memset(vaug[:, :, :, DH:128], 1.0)
        saT = const.tile([128, KT, N], bf16)  # sa^T, local d_in rows
        nc.scalar.activation(saT[:, 0, 0:1], dum0, EXP)
        # bacc pre-registers const APs; the BIR verifier rejects unread
        # SBUF - give the unused ones readers (slots overwritten later)
        for i, key in enumerate([(f32, 1.0), (bf16, 1.0),
                                 (mybir.dt.uint8, 127)]):
            nc.vector.tensor_copy(saT[:, 0, i + 1:i + 2],
                                  nc.const_aps.aps[key])
        # identity for PE-mode transpose
        ones = const.tile([128, 128], bf16)
        nc.gpsimd.memset(ones, 1.0)
        ident = const.tile([128, 128], bf16)
        nc.gpsimd.affine_select(
            ident, ones, pattern=[[-1, 128]], base=0,
            channel_multiplier=1,
            compare_op=mybir.AluOpType.is_equal, fill=0.0)

        # ---------------- input DMA schedule ----------------
        # (GpSimd cannot touch PSUM on TRN2: all psum moves are DVE/ACT)
        # gpsimd ring: weights + bias; sync ring: x dt-pairs 0,2 (+ out
        # DMAs later); scalar ring: x dt-pairs 1,3.
        nc.gpsimd.dma_start(out=wst[:, 0:3], in_=wTr[:, 0:3])
        nc.gpsimd.dma_start(out=wv16[:, 0:3], in_=wv16r[:, 0:3])
        for dtp in range(DP):
            eng = nc.sync if dtp % 2 == 0 else nc.scalar
            eng.dma_start(out=xT[:, 2 * dtp:2 * dtp + 2, :],
                          in_=xTr[:, 2 * dtp:2 * dtp + 2, :])
            eng.dma_start(out=x16[:, 2 * dtp:2 * dtp + 2, :],
                          in_=x16r[:, 2 * dtp:2 * dtp + 2, :])
        nc.gpsimd.dma_start(out=bq, in_=bq_d[:, :])

        def fetch_aux_w():
            nc.gpsimd.dma_start(out=wst[:, 3:6], in_=wTr[:, 3:6])
            nc.gpsimd.dma_start(out=wv16[:, 3:6], in_=wv16r[:, 3:6])

        def fetch_wpT():
            nc.gpsimd.dma_start(out=wpT, in_=wpTr)

        # ---------------- QKV building blocks ----------------
        # e-slab -> destination: v->vsrc, k/q->kqv
        def qkv_dst(et):
            return {0: vsrc[:, 0, :], 1: kqv[:, 0, :], 2: kqv[:, 1, :],
                    3: vsrc[:, 1, :], 4: kqv[:, 2, :], 5: kqv[:, 3, :]}[et]

        def qkv_mm(et, nbs, dtp, pss):
            for slot, nb in enumerate(nbs):
                if nb == 0:
                    # n[0:512] in bf16: 2 plain matmuls per dt-pair
                    for dt in (2 * dtp, 2 * dtp + 1):
                        nc.tensor.matmul(
                            pss[slot],
                            lhsT=wv16[:, et, dt, :],
                            rhs=x16[:, dt, :],
                            start=(dt == 0),
                            stop=(dt == DT - 1),
                        )
                    continue
                nc.tensor.matmul(
                    pss[slot],
                    lhsT=wst[:, et, 2 * dtp:2 * dtp + 2, :],
                    rhs=xT[:, 2 * dtp:2 * dtp + 2,
                           nb * NBS:(nb + 1) * NBS],
                    start=(dtp == 0),
                    stop=(dtp == DP - 1),
                    perf_mode=DR,
                )

        def qkv_bias(et, nb, ps):
            # psum holds 16*(W@x); rescale and add bias (DVE, reads psum)
            nc.vector.tensor_scalar(
                out=qkv_dst(et)[:, nb * NBS:(nb + 1) * NBS],
                in0=ps,
                scalar1=1.0 / 16.0,
                scalar2=bq[:, et:et + 1],
                op0=MULT, op1=ADD,
            )

        def qkv_group_items(et):
            """Closures for one e-slab (fp8 n-blocks first, bf16 last)."""
            items = []
            for nbs in ((1, 2), (3, 0)):
                holder = {}

                def alloc(et=et, nbs=nbs, holder=holder):
                    holder["pss"] = [
                        qps.tile([128, NBS], f32, tag="qkvps",
                                 name=f"qkvps{et}_{i}")
                        for i in range(2)]
                items.append(alloc)
                for dtp in range(DP):
                    items.append(lambda et=et, nbs=nbs, dtp=dtp,
                                 holder=holder:
                                 qkv_mm(et, nbs, dtp, holder["pss"]))
                for i, nb in enumerate(nbs):
                    items.append(lambda et=et, nb=nb, i=i, holder=holder:
                                 qkv_bias(et, nb, holder["pss"][i]))
            return items

        def vtrans_items(vt):
            """V transpose on the PE + per-head copies into vaug(16).

            NOTE: each item allocates a psum tile from the qkv tag; items
            must only run at qkv-group boundaries (aux list order
            guarantees this) or the in-order PE queue deadlocks waiting
            for a slot freed by a later instruction.
            """
            items = []

            def tr(mt, vt=vt):
                pv = qps.tile([128, 128], bf16, tag="qkvps", name="pv")
                with nc.allow_low_precision(reason="transpose pass"):
                    nc.tensor.transpose(
                        pv, vsrc[:, vt, mt * MTS:(mt + 1) * MTS], ident)
                nc.scalar.copy(vaug[:, 2 * vt:2 * vt + 2, mt, 0:DH], pv)
                if mt < 4:
                    nc.vector.tensor_copy(
                        vaug16[:, 2 * vt:2 * vt + 2, mt, 0:DH], pv)

            for mt0 in range(0, MT, 2):
                def grp(mt0=mt0, vt=vt):
                    tr(mt0, vt)
                    tr(mt0 + 1, vt)
                items.append(grp)
            return items

        # ---------------- serial prefix: v01, k01, q01 ----------------
        for it in qkv_group_items(0):
            it()
        for it in vtrans_items(0)[:2]:      # vaug mts 0..3 for first PVs
            it()
        for et in (1, 2):
            for it in qkv_group_items(et):
                it()

        # aux work paced into the attention pipeline.  Transpose items
        # sit between complete qkv groups (see vtrans_items note).
        aux = vtrans_items(0)[2:]
        aux.append(fetch_aux_w)
        aux.extend(qkv_group_items(3))
        aux.extend(vtrans_items(1))
        aux.extend(qkv_group_items(4))
        aux.extend(qkv_group_items(5))
        aux.append(fetch_wpT)

        # ---------------- attention + projection pipeline ----------------
        # unit = (head-pair, q-block); g = (unit, m-tile)
        units = []
        for hp in (0, 1):
            for qb in (3, 2, 1, 0):
                units.append((hp, qb))
        gp = []
        for ui, (hp, qb) in enumerate(units):
            for mt in range(4 * qb + 4):
                gp.append((ui, mt))
        TOT = len(gp)                 # 80

        sap_tiles = {}
        ptp_tiles = {}
        state = {"s": 0}

        def emit_S(g):
            ui, mt = gp[g]
            hp, qb = units[ui]
            qsl = slice(qb * NBS, (qb + 1) * NBS)
            msl = slice(mt * MTS, (mt + 1) * MTS)
            diag = mt >= 4 * qb
            qlo = (mt - 4 * qb) * MTS if diag else 0
            if mt % 2 == 0:
                # qb=0 in bf16 (exact few-key rows), others fp8 for PV-DR
                pdt, ptag = (bf16, "ptb") if qb == 0 else (f8, "pt")
                ptp_tiles[g] = pts.tile([128, 2, 2, NBS], pdt, tag=ptag,
                                        name="ptp")
            ptp = ptp_tiles[g - (mt % 2)]
            s = mt % 2
            sp = sps.tile([128, 2, NBS], f32, tag="sp", name="sp")
            for j in range(2):
                pb = slice(64 * j, 64 * j + 64)
                nc.tensor.matmul(
                    sp[:, j, qlo:],
                    lhsT=kqv[pb, 2 * hp, msl],
                    rhs=kqv[pb, 2 * hp + 1, qsl][:, qlo:],
                    start=True, stop=True,
                )
            if not diag:
                nc.scalar.activation(ptp[:, s, :, :], sp, EXP, scale=0.125,
                                     bias=nbias)
            else:
                if qlo + MTS < NBS:
                    nc.scalar.activation(ptp[:, s, :, qlo + MTS:],
                                         sp[:, :, qlo + MTS:],
                                         EXP, scale=0.125, bias=nbias)
                sdt, stag = (bf16, "scrb") if qb == 0 else (f8, "scr")
                scr = pts.tile([128, 2, MTS], sdt, tag=stag, name="scr")
                nc.scalar.activation(scr, sp[:, :, qlo:qlo + MTS],
                                     EXP, scale=0.125, bias=nbias)
                # causal staircase on GpSimd: keep where q-col >= m-row
                nc.gpsimd.affine_select(
                    ptp[:, s, :, qlo:qlo + MTS], scr,
                    pattern=[[0, 2], [1, MTS]], base=0,
                    channel_multiplier=-1,
                    compare_op=mybir.AluOpType.is_ge, fill=0.0)
                if s == 1 and qb != 0:
                    # zero the strip the PV DoubleRow pair will read
                    # below this tile's causal start
                    nc.gpsimd.memset(ptp[:, 1, :, qlo - MTS:qlo], 0.0)

        def pump_S(upto):
            while state["s"] <= min(upto, TOT - 1):
                emit_S(state["s"])
                state["s"] += 1

        def emit_PV(g):
            ui, mt = gp[g]
            hp, qb = units[ui]
            diag0 = (mt - 1) >= 4 * qb
            qloE = (mt - 1 - 4 * qb) * MTS if diag0 else 0
            first = (mt == 1)
            last = (mt == 4 * qb + 3)
            ptp = ptp_tiles[g - 1]
            if first:
                sap_tiles[ui] = sapp.tile([128, 2, NBS], f32, tag="sap",
                                          name="sap")
            if qb == 0:
                # bf16 PV, one matmul per m-tile with exact causal range
                for j in range(2):
                    for m in (mt - 1, mt):
                        qlo = m * MTS
                        nc.tensor.matmul(
                            sap_tiles[ui][:, j, qlo:],
                            lhsT=vaug16[:, 2 * hp + j, m, :],
                            rhs=ptp[:, m % 2, j, qlo:],
                            start=(m == 0), stop=(m == 3),
                        )
            else:
                for j in range(2):
                    nc.tensor.matmul(
                        sap_tiles[ui][:, j, qloE:],
                        lhsT=vaug[:, 2 * hp + j, mt - 1:mt + 1, :],
                        rhs=ptp[:, 0:2, j, qloE:],
                        start=first, stop=last,
                        perf_mode=DR,
                    )
            del ptp_tiles[g - 1]

        def emit_norm(ui):
            hp, qb = units[ui]
            qsl = slice(qb * NBS, (qb + 1) * NBS)
            for j in range(2):
                sap = sap_tiles[ui][:, j, :]
                # HW constraints (micro-tested): reciprocal_approx_fast
                # only works at base partition 0, and 2-input DVE ops
                # need equal input base partitions.
                den = rrp.tile([128, NBS], f32, tag="den", name="den")
                nc.vector.tensor_copy(den[0:DH, :], sap[DH:128, :])
                rr = rrp.tile([128, NBS], f32, tag="rr", name="rr")
                nc.vector.reciprocal_approx_fast(
                    out=rr[0:DH, :], in_=den[0:DH, :])
                nc.vector.tensor_mul(
                    saT[64 * j:64 * j + 64, hp, qsl],
                    sap[0:DH, :], rr[0:DH, :])
            del sap_tiles[ui]

        def emit_proj(nt, borrow_sp=False):
            nsl = slice(nt * 128, (nt + 1) * 128)
            if borrow_sp:
                po = sps.tile([128, 2, NBS], f32, tag="sp", name="sp")
                po0, po1 = po[:, 0, :], po[:, 1, :]
            else:
                po0 = qps.tile([128, NBS], f32, tag="qkvps", name="po0")
                po1 = qps.tile([128, NBS], f32, tag="qkvps", name="po1")
            for kt in range(KT):
                lt = saT[:, kt, nsl]
                nc.tensor.matmul(po0, lhsT=lt, rhs=wpT[:, kt, 0:NBS],
                                 start=(kt == 0), stop=(kt == KT - 1))
                nc.tensor.matmul(po1, lhsT=lt, rhs=wpT[:, kt, NBS:D],
                                 start=(kt == 0), stop=(kt == KT - 1))
            ot = ost.tile([128, D], bf16, name="ot")
            nc.vector.tensor_copy(ot[:, 0:NBS], po0)
            nc.vector.tensor_copy(ot[:, NBS:], po1)
            nc.sync.dma_start(out=out_d[nsl, :], in_=ot)

        pump_S(1)
        for g in range(TOT):
            ui, mt = gp[g]
            hp, qb = units[ui]
            pump_S(g + 2)
            budget = 1
            if mt % 2 == 1:
                emit_PV(g)
                budget = 2
            while aux and budget > 0:
                aux.pop(0)()
                budget -= 1
            if mt == 4 * qb + 3:      # last m-tile of this unit
                emit_norm(ui)
                if hp == 1 and qb != 0:   # qb's saT slab is complete
                    aux.extend(
                        lambda nt=4 * qb + k: emit_proj(nt)
                        for k in range(4))
        while aux:
            aux.pop(0)()
        # tail: proj for the last finished qb (qb=0), borrowing the
        # drained score psum for buffering
        for k in range(4):
            emit_proj(k, borrow_sp=True)

    nc.compile()
    return nc


def _host_inputs(x, Wqkv, bqkv, Wproj):
    """Per-core input maps (host-side sharding + relayout, fp8/bf16 cast).

    All tensors are packed partition-major so every DMA descriptor is a
    contiguous row chunk.  fp8 weights are pre-scaled by 16 so e4m3
    keeps mantissa bits (values ~0.3 land mid-range); the kernel
    rescales by 1/16 during psum->SBUF moves (the bf16 wv16 copies get
    the same scale so both paths share the bias-move).
    """
    import ml_dtypes
    f8 = ml_dtypes.float8_e4m3
    bf16 = ml_dtypes.bfloat16

    in_maps = []
    for c in range(8):
        b, hg = c // NH, c % NH
        h0 = hg * NH
        # xT[p, dt, n] = x[b][n, dt*128+p]
        xT = np.ascontiguousarray(
            x[b].T.reshape(DT, 128, N).transpose(1, 0, 2)
            .reshape(128, DT * N)).astype(f8)
        # e-slab order [v01 k01 q01 | v23 k23 q23]; within a slab the
        # head-pair's even head occupies rows 0:64, odd head 64:128.
        wq = Wqkv[h0:h0 + NH].reshape(NH, 3, DH, D)
        slabs = []
        bslabs = []
        bqc = bqkv[h0:h0 + NH].reshape(NH, 3, DH)
        for p2 in (0, 1):
            for comp in (2, 0, 1):            # v, k, q
                slabs.append(wq[2 * p2:2 * p2 + 2, comp].reshape(128, D))
                bslabs.append(bqc[2 * p2:2 * p2 + 2, comp].reshape(128))
        wre = np.concatenate(slabs, 0)        # [768, D]
        wT = (wre * 16.0).T                   # [D, 768]
        # wT2[p, et, dt, j] = wT[dt*128+p, et*128+j]
        wT2 = np.ascontiguousarray(
            wT.reshape(DT, 128, ET, 128).transpose(1, 2, 0, 3)
            .reshape(128, ET * DT * 128)).astype(f8)
        bq2 = np.ascontiguousarray(
            np.stack(bslabs, axis=1)).astype(np.float32)   # [128, ET]
        wpT = Wproj[:, h0 * DH:(h0 + NH) * DH].T           # [256, D]
        wpT2 = np.ascontiguousarray(
            wpT.reshape(KT, 128, D).transpose(1, 0, 2)
            .reshape(128, KT * D)).astype(bf16)
        x16 = np.ascontiguousarray(
            x[b][0:NBS, :].T.reshape(DT, 128, NBS).transpose(1, 0, 2)
            .reshape(128, DT * NBS)).astype(bf16)
        # all slabs in bf16 (for the n[0:512] block), same pre-scale
        vsl = np.stack(slabs, 0) * 16.0                  # [ET, 128, D]
        wv16 = np.ascontiguousarray(
            vsl.transpose(0, 2, 1).reshape(ET, DT, 128, 128)
            .transpose(2, 0, 1, 3).reshape(128, ET * DT * 128)
        ).astype(bf16)
        in_maps.append({"xT": xT, "wT": wT2, "bq": bq2, "wpT": wpT2,
                        "x16": x16, "wv16": wv16})
    return in_maps


def _get_nc():
    if "nc" not in _CACHE:
        _CACHE["nc"] = _build_nc()
    return _CACHE["nc"]


def run_on_hw(in_maps, trace=False, **kw):
    from concourse.bass_utils import run_bass_kernel_spmd
    nc = _get_nc()
    return run_bass_kernel_spmd(
        nc, in_maps, core_ids=list(range(8)), trace=trace, **kw)


def kernel(**inputs):
    x = np.asarray(inputs["x"], dtype=np.float32)
    Wqkv = np.asarray(inputs["Wqkv"], dtype=np.float32)
    bqkv = np.asarray(inputs["bqkv"], dtype=np.float32)
    Wproj = np.asarray(inputs["Wproj"], dtype=np.float32)
    bproj = np.asarray(inputs["bproj"], dtype=np.float32)

    in_maps = _host_inputs(x, Wqkv, bqkv, Wproj)
    res = run_on_hw(in_maps).results

    out = np.zeros((B, N, D), dtype=np.float32)
    for b in range(B):
        acc = res[b * NH + 0]["outp"].astype(np.float32)
        for g in range(1, NH):
            acc = acc + res[b * NH + g]["outp"].astype(np.float32)
        out[b] = acc + bproj[None, :]
    return out
